# revision 15
# baseline (speedup 1.0000x reference)
"""Trainium2 Bass kernel for nn_Net_49177375539428 (gnn_message_passing).

Strategy (see schedule builder below):
  - One core per candidate graph (8 graphs, 8 NeuronCores), single SPMD
    program with an 8-way switch on partition id; each branch is fully
    specialized to its graph's tree.
  - The (E,D) embedding matrices of the reference are row-constant except on
    the ancestor path of `pos`, so the computation decomposes into
      * a scalar chain: one vector x matrix transform per node (LDW + 1-col
        matmul accumulating straight into the parent's PSUM column,
        transposed layout [d, node]),
      * a branch at `pos` over all E edge matrices (1-col matmuls into a
        [d, e] PSUM tile, sharing weight loads with the chain),
      * ~log N full (D,D) matmuls along the pos->root path.
  - Edge weights are shipped once per core (8 MB) in first-use order and
    streamed through SBUF in chunks so DMA overlaps the PE work.
"""

import os
import numpy as np

import concourse.bass as bass
import concourse.mybir as mybir
import concourse.tile as tile
from concourse import bacc
from concourse.bass_utils import run_bass_kernel_spmd

N = 128          # nodes per graph
E = 128          # edge types
D = 128          # embedding dim
G = 8            # graphs / cores
VEC = 300        # word-vec dim
VEC_PAD = 384    # padded to 3x128
NCOLS = 132      # EMB columns: 128 nodes + pos_pure + pad to 4
POS_PURE_COL = 128
W_CHUNK_SLOTS = 32         # edge matrices per DMA chunk
N_W_CHUNKS = E // W_CHUNK_SLOTS

F32 = mybir.dt.float32
F16 = mybir.dt.float16

LAST_RESULT = None         # BassKernelResults of the most recent run


# ----------------------------------------------------------------------------
# Host-side schedule construction
# ----------------------------------------------------------------------------

class GraphSchedule:
    """Per-graph specialization: column assignment, wave structure, matmul
    schedule entries, and the per-core data (weight order, gvT, Mult)."""

    def __init__(self, g_row, edges, pos):
        parents = np.empty(N, np.int64)
        for i in range(N - 1):
            parents[i] = i + int(g_row[i])
        parents[N - 1] = -1
        children = [[] for _ in range(N)]
        for i in range(N - 1):
            children[parents[i]].append(i)
        internal = np.array([len(children[n]) > 0 for n in range(N)])
        depth = np.zeros(N, np.int64)
        for i in range(N - 2, -1, -1):
            depth[i] = depth[parents[i]] + 1
        maxdepth = int(depth.max())

        assert pos != N - 1, "pos == root not supported"
        path = []
        n = pos
        while n != N - 1:
            n = parents[n]
            path.append(n)
        pathset = set(path)

        # subtree of pos (incl. pos)
        sub = set()
        stack = [pos]
        while stack:
            n = stack.pop()
            sub.add(n)
            stack.extend(children[n])

        # Column assignment, level-major.  Within each level:
        #   [subtree-internal | other-internal | path nodes | leaves]
        col = np.full(N, -1, np.int64)
        self.sub_int_range = {}   # lvl -> (start, end)
        self.oth_int_range = {}   # lvl -> (start, end)
        off = 0
        lvl_nodes = [[] for _ in range(maxdepth + 1)]
        for n in range(N):
            lvl_nodes[depth[n]].append(n)
        for lvl in range(maxdepth + 1):
            nodes = lvl_nodes[lvl]
            sub_int = [n for n in nodes if n in sub and internal[n]]
            oth_int = [n for n in nodes
                       if internal[n] and n not in sub and n not in pathset]
            pth = [n for n in nodes if n in pathset]
            leaves = [n for n in nodes if not internal[n] and n not in pathset]
            self.sub_int_range[lvl] = (off, off + len(sub_int))
            for n in sub_int:
                col[n] = off
                off += 1
            self.oth_int_range[lvl] = (off, off + len(oth_int))
            for n in oth_int:
                col[n] = off
                off += 1
            for n in pth:
                col[n] = off
                off += 1
            for n in leaves:
                col[n] = off
                off += 1
        assert off == N

        self.parents, self.children = parents, children
        self.internal, self.depth, self.maxdepth = internal, depth, maxdepth
        self.path, self.pathset, self.sub = path, pathset, sub
        self.col = col
        self.pos = pos
        self.edges = edges
        self.path_idx = {a: k for k, a in enumerate(path)}
        # does path node have any chain (non-path, non-pos) children?
        self.path_has_chain = {
            a: any((c not in pathset) and c != pos for c in children[a])
            for a in path
        }

        self._build_entries()
        self._build_data_tables()

    def _build_entries(self):
        """Entries: (edge, [(src_col, psum_name, dst_col, start, stop)]).
        psum tiles: 'mini{lvl}', 'wave{lvl}', 'path', 'branch'."""
        edges, children, depth = self.edges, self.children, self.depth
        pos, sub, pathset = self.pos, self.sub, self.pathset
        col = self.col

        entries = []          # list of (edge_id, mm list)
        self.finalizes = []   # (after_entry_index, psum_name, psum_lo, psum_hi,
                              #  emb_lo, emb_hi)  -> EMB[lo:hi] = relu(psum+EMB)
        self.psum_sizes = {}

        # start/stop bookkeeping per (psum_name, dst_col)
        first_write = {}

        def add_wave(kids_by_edge, psum_name, dst_of, branch_ok):
            """kids grouped per edge; returns nothing, appends entries."""
            # count writers per dst col for stop flags
            writer_cnt = {}
            for e, kids in kids_by_edge.items():
                for c in kids:
                    d = dst_of(c)
                    writer_cnt[d] = writer_cnt.get(d, 0) + 1
            seen_cnt = {}
            for e in sorted(kids_by_edge):
                mms = []
                for c in kids_by_edge[e]:
                    d = dst_of(c)
                    seen_cnt[d] = seen_cnt.get(d, 0) + 1
                    key = (psum_name, d)
                    start = key not in first_write
                    first_write[key] = True
                    # stop on last writer of this column *within this wave*;
                    # (for 'path' tile, columns accumulate across waves: stop
                    # only when total count reached — handled via total counts)
                    stop = seen_cnt[d] == writer_cnt[d]
                    mms.append((col[c] if c >= 0 else POS_PURE_COL,
                                psum_name, d, start, stop))
                if branch_ok and not self.branch_done[e]:
                    self.branch_done[e] = True
                    mms.append((col[pos], 'branch', e, True, True))
                entries.append((e, mms))

        self.branch_done = [False] * E

        # --- mini waves: subtree of pos, deepest level first -----------------
        sub_lvls = sorted({int(depth[n]) for n in sub}, reverse=True)
        for lvl in sub_lvls:
            # children at this level whose parents are in sub at lvl-1
            kids = [n for n in sub if depth[n] == lvl and n != pos]
            if kids:
                by_edge = {}
                for c in kids:
                    by_edge.setdefault(int(edges[c]), []).append(c)
                add_wave(by_edge, f'mini{lvl}',
                         lambda c: col[self.parents[c]]
                         - self.sub_int_range[int(depth[c]) - 1][0],
                         branch_ok=False)
                lo, hi = self.sub_int_range[lvl - 1]
                self.psum_sizes[f'mini{lvl}'] = hi - lo
                self.finalizes.append((len(entries), f'mini{lvl}',
                                       0, hi - lo, lo, hi))
        # v_pos is now available (pos is leaf, or finalized by last mini wave)

        # --- main waves ------------------------------------------------------
        for lvl in range(self.maxdepth, 0, -1):
            kids = [n for n in range(N)
                    if depth[n] == lvl and n not in sub and n not in pathset]
            olo, ohi = self.oth_int_range[lvl - 1]
            if kids:
                by_edge = {}
                for c in kids:
                    by_edge.setdefault(int(edges[c]), []).append(c)

                def dst_of(c):
                    p = self.parents[c]
                    if p in pathset:
                        return ('path', self.path_idx[p])
                    return (f'wave{lvl - 1}', col[p] - olo)

                wcnt = {}
                for e, kids_e in by_edge.items():
                    for c in kids_e:
                        tgt = dst_of(c)
                        wcnt[tgt] = wcnt.get(tgt, 0) + 1
                seen = {}
                for e in sorted(by_edge):
                    mms = []
                    for c in by_edge[e]:
                        name, d = dst_of(c)
                        tgt = (name, d)
                        seen[tgt] = seen.get(tgt, 0) + 1
                        start = tgt not in first_write
                        first_write[tgt] = True
                        stop = seen[tgt] == wcnt[tgt]
                        mms.append((col[c], name, d, start, stop))
                    if not self.branch_done[e]:
                        self.branch_done[e] = True
                        mms.append((col[pos], 'branch', e, True, True))
                    entries.append((e, mms))
            if ohi > olo:
                self.psum_sizes[f'wave{lvl - 1}'] = ohi - olo
                self.finalizes.append((len(entries), f'wave{lvl - 1}',
                                       0, ohi - olo, olo, ohi))

        # --- leftover branch edges ------------------------------------------
        for e in range(E):
            if not self.branch_done[e]:
                self.branch_done[e] = True
                entries.append((e, [(self.col[pos], 'branch', e, True, True)]))

        self.psum_sizes['branch'] = E
        self.psum_sizes['path'] = max(1, len(self.path))

        # PSUM start/stop semantics: start=True lazily zeroes the ENTIRE
        # 2KB bank (pending-zero), after which the per-byte pending flag
        # makes fresh columns overwrite and touched columns accumulate.
        # So: start only on the very first matmul into each tile, stop on
        # the last.  (Per-column start flags would wipe sibling columns.)
        totals = {}
        for _, mms in entries:
            for (_, pname, _, _, _) in mms:
                totals[pname] = totals.get(pname, 0) + 1
        seen = {}
        fixed = []
        for e, mms in entries:
            new_mms = []
            for (src, pname, dst, _, _) in mms:
                k = seen.get(pname, 0)
                seen[pname] = k + 1
                new_mms.append((src, pname, dst,
                                k == 0, k + 1 == totals[pname]))
            fixed.append((e, new_mms))
        self.entries = fixed

        # Wbuf slot order = first use
        slot_of = {}
        for e, _ in entries:
            if e not in slot_of:
                slot_of[e] = len(slot_of)
        assert len(slot_of) == E
        self.slot_of = slot_of

    def _build_data_tables(self):
        """Per-core numpy inputs: Wbuf (first-use order), Mult, perm for gvT."""
        perm = np.empty(E, np.int64)     # slot -> edge id
        for e, s in self.slot_of.items():
            perm[s] = e
        self.w_perm = perm

        # multiplicity matrix: Mult[e, col(p)] = # chain children of p with edge e
        mult = np.zeros((E, NCOLS), np.float32)
        for p in range(N):
            for c in self.children[p]:
                if c in self.pathset or c == self.pos:
                    continue
                mult[int(self.edges[c]), self.col[p]] += 1.0
        self.mult = mult


# ----------------------------------------------------------------------------
# Bass program
# ----------------------------------------------------------------------------

def _build_program(scheds):
    nc = bacc.Bacc("TRN2", target_bir_lowering=False, debug=False, num_devices=G)

    # shared inputs
    t_dw = nc.declare_dram_parameter("dw", [3, 128, D], F16, isOutput=False)
    t_eb = nc.declare_dram_parameter("eb", [E, D], F16, isOutput=False)
    t_ebT = nc.declare_dram_parameter("ebT", [D, E], F32, isOutput=False)
    t_db = nc.declare_dram_parameter("db", [D, 1], F32, isOutput=False)
    t_scw = nc.declare_dram_parameter("scw", [D, 2], F32, isOutput=False)
    t_sb = nc.declare_dram_parameter("sb", [1, 1], F32, isOutput=False)
    # per-core inputs
    t_gvT = nc.declare_dram_parameter("gvT", [3, 128, NCOLS], F16, isOutput=False)
    t_mult = nc.declare_dram_parameter("mult", [E, NCOLS], F16, isOutput=False)
    t_w = nc.declare_dram_parameter("wbuf", [D, E * D], F16, isOutput=False)
    t_out = nc.declare_dram_parameter("scores", [1, E], F32, isOutput=True)

    with tile.TileContext(nc) as tc:
        with (
            tc.tile_pool(name="wpool", bufs=1) as wpool,
            tc.tile_pool(name="sbuf", bufs=1) as pool,
            tc.tile_pool(name="ppool", bufs=2, space="PSUM") as ppool,
            tc.tile_pool(name="ppool_fix", bufs=1, space="PSUM") as ppool_fix,
        ):
            # All DMA loads are identical instructions across graphs (per-core
            # content differs via in_maps) — issue them before the Switch so
            # transfers stream from t=0, overlapping dispatch + compute.
            # W chunks first, all on the sync HWDGE: triggers land at t~0
            # and the hardware DGE streams the 4.2MB autonomously.
            wb = t_w.ap().rearrange("p (c s) -> p c s", c=N_W_CHUNKS)
            w_chunks = []
            for c in range(N_W_CHUNKS):
                wt = wpool.tile([D, W_CHUNK_SLOTS * D], F16, tag=f"w{c}",
                                name=f"w{c}")
                nc.sync.dma_start(wt[:], wb[:, c, :])
                w_chunks.append(wt)
            sb_tiles = {}
            sb_tiles['dw'] = pool.tile([128, 3, D], F16, tag="dw", name="dw")
            nc.scalar.dma_start(sb_tiles['dw'][:], t_dw.ap().rearrange("c p d -> p c d"))
            sb_tiles['gv'] = pool.tile([128, 3, NCOLS], F16, tag="gv", name="gv")
            nc.scalar.dma_start(sb_tiles['gv'][:], t_gvT.ap().rearrange("c p d -> p c d"))
            sb_tiles['mult'] = pool.tile([E, NCOLS], F16, tag="mult", name="mult")
            nc.scalar.dma_start(sb_tiles['mult'][:], t_mult[:])
            sb_tiles['eb'] = pool.tile([E, D], F16, tag="eb", name="eb")
            nc.scalar.dma_start(sb_tiles['eb'][:], t_eb[:])
            sb_tiles['ebT'] = pool.tile([D, E], F32, tag="ebT", name="ebT")
            nc.scalar.dma_start(sb_tiles['ebT'][:], t_ebT[:])
            sb_tiles['db'] = pool.tile([D, 1], F32, tag="db", name="db")
            nc.scalar.dma_start(sb_tiles['db'][:], t_db[:])
            sb_tiles['scw'] = pool.tile([D, 2], F32, tag="scw", name="scw")
            nc.scalar.dma_start(sb_tiles['scw'][:], t_scw[:])
            sb_tiles['sb'] = pool.tile([1, 1], F32, tag="sb", name="sb")
            nc.scalar.dma_start(sb_tiles['sb'][:], t_sb[:])
            pid = nc.partition_id()
            for j in tc.Switch(pid, G):
                _emit_graph(nc, scheds[j], pool, ppool, ppool_fix,
                            sb_tiles, w_chunks, t_out)
    nc.finalize()
    return nc


def _emit_graph(nc, S, pool, ppool, ppool_fix, sb_tiles, w_chunks, t_out):
    Relu = mybir.ActivationFunctionType.Relu
    ADD = mybir.AluOpType.add
    dw_sb = sb_tiles['dw']
    eb_sb = sb_tiles['eb']
    ebT_sb = sb_tiles['ebT']
    db_sb = sb_tiles['db']
    scw_sb = sb_tiles['scw']
    sb_sb = sb_tiles['sb']
    gv_sb = sb_tiles['gv']
    mult_sb = sb_tiles['mult']

    def w_ap(edge):
        s = S.slot_of[edge]
        return w_chunks[s // W_CHUNK_SLOTS][
            :, (s % W_CHUNK_SLOTS) * D:(s % W_CHUNK_SLOTS + 1) * D]

    # ---- EMB = baseB (base + bias-sums), then finalized in-place ----
    ps_base = ppool_fix.tile([128, NCOLS], F32, tag="ps_base")
    for k in range(3):
        nc.tensor.matmul(ps_base[:], dw_sb[:, k, :], gv_sb[:, k, :],
                         start=(k == 0), stop=False)
    nc.tensor.matmul(ps_base[:], eb_sb[:], mult_sb[:], start=False, stop=True)
    emb = pool.tile([128, NCOLS], F32, tag="emb")
    nc.vector.tensor_scalar(emb[:], ps_base[:], db_sb[:, 0:1], None, ADD)
    emb16 = pool.tile([128, NCOLS], F16, tag="emb16")
    nc.vector.tensor_copy(emb16[:], emb[:])

    # ---- psum tiles for waves / branch / path ----
    ps = {}
    ps['branch'] = ppool_fix.tile([128, E], F32, tag="ps_branch", name="ps_branch")
    ps['path'] = ppool_fix.tile([128, S.psum_sizes['path']], F32, tag="ps_path", name="ps_path")
    for name, sz in S.psum_sizes.items():
        if name in ('branch', 'path'):
            continue
        ps[name] = ppool.tile([128, sz], F32, tag="ps_wave", name=f"ps_{name}")

    # ---- chain + branch matmuls with interleaved finalizes ----
    fin = list(S.finalizes)
    fi = 0
    for idx, (e, mms) in enumerate(S.entries):
        while fi < len(fin) and fin[fi][0] == idx:
            _finalize(nc, emb, emb16, ps, fin[fi])
            fi += 1
        wap = w_ap(e)
        for (src, pname, dst, start, stop) in mms:
            nc.tensor.matmul(ps[pname][:, dst:dst + 1], wap,
                             emb16[:, src:src + 1], start=start, stop=stop)
    while fi < len(fin):
        _finalize(nc, emb, emb16, ps, fin[fi])
        fi += 1

    # ---- path walk ----
    # step k at ancestor a_k:  M_k = relu(prev_transform + b_{e_{k-1}} + u_k)
    # with u_k = baseB[a_k] (+ chain psum).  Bias columns are precomputed so
    # each step is ONE fused DVE op (add per-partition bias, max 0) + one MM.
    plen = len(S.path)
    biases = pool.tile([128, max(plen, 1)], F32, tag="biases")
    for k, a in enumerate(S.path):
        acol = int(S.col[a])
        eprev = None if k == 0 else int(S.edges[S.path[k - 1]])
        if S.path_has_chain[a]:
            nc.vector.tensor_tensor(
                biases[:, k:k + 1], ps['path'][:, k:k + 1],
                emb[:, acol:acol + 1], ADD)
            if eprev is not None:
                nc.vector.tensor_tensor(
                    biases[:, k:k + 1], biases[:, k:k + 1],
                    ebT_sb[:, eprev:eprev + 1], ADD)
        elif eprev is not None:
            nc.vector.tensor_tensor(
                biases[:, k:k + 1], emb[:, acol:acol + 1],
                ebT_sb[:, eprev:eprev + 1], ADD)
        else:
            nc.vector.tensor_copy(biases[:, k:k + 1], emb[:, acol:acol + 1])

    mcur = pool.tile([128, E], F32, tag="mcur")
    mnext = pool.tile([128, E], F16, tag="mnext")
    cur_ps = ps['branch']
    for k, a in enumerate(S.path):
        if k == 0:
            # branch contribution carries full per-column edge biases
            nc.vector.tensor_tensor(mcur[:], cur_ps[:], ebT_sb[:], ADD)
            nc.vector.tensor_scalar(mnext[:], mcur[:], biases[:, 0:1], 0.0,
                                    ADD, mybir.AluOpType.max)
        else:
            nc.vector.tensor_scalar(mnext[:], cur_ps[:], biases[:, k:k + 1],
                                    0.0, ADD, mybir.AluOpType.max)
        ea = int(S.edges[a])
        ps_step = ppool_fix.tile([128, E], F32, tag="ps_step")
        nc.tensor.matmul(ps_step[:], w_ap(ea), mnext[:], start=True, stop=True)
        cur_ps = ps_step

    # res^T = ps_step + b_eroot  [d, e]
    eroot = int(S.edges[N - 1])
    nc.vector.tensor_scalar(mcur[:], cur_ps[:],
                            ebT_sb[:, eroot:eroot + 1], None, ADD)
    ps_sc = ppool_fix.tile([1, E + 4], F32, tag="ps_sc")
    nc.tensor.matmul(ps_sc[:, 0:E], scw_sb[:, 0:1], mcur[:],
                     start=True, stop=False)
    nc.tensor.matmul(ps_sc[:, E:E + 1], scw_sb[:, 1:2],
                     emb[:, POS_PURE_COL:POS_PURE_COL + 1], start=False, stop=True)
    dsc = pool.tile([1, 1], F32, tag="dsc")
    nc.vector.tensor_tensor(dsc[:], ps_sc[:, E:E + 1], sb_sb[:], ADD)
    srow = pool.tile([1, E], F32, tag="srow")
    nc.vector.tensor_scalar(srow[:], ps_sc[:, 0:E], dsc[:], None, ADD)
    nc.sync.dma_start(t_out[:], srow[:])


def _finalize(nc, emb, emb16, ps, f):
    _, name, plo, phi, elo, ehi = f
    if phi <= plo:
        return
    nc.vector.tensor_tensor(emb[:, elo:ehi], ps[name][:, plo:phi],
                            emb[:, elo:ehi], mybir.AluOpType.add)
    nc.vector.tensor_scalar(emb16[:, elo:ehi], emb[:, elo:ehi],
                            0.0, None, mybir.AluOpType.max)


# ----------------------------------------------------------------------------
# Host entry point
# ----------------------------------------------------------------------------

def kernel(**inputs):
    global LAST_RESULT
    data = np.asarray(inputs["data"])
    graphs = np.asarray(inputs["graphs"])
    edges = np.asarray(inputs["edges"])
    pos = int(np.asarray(inputs["pos"]))
    dv = np.asarray(inputs["data_vecs"], dtype=np.float32)
    dw = np.asarray(inputs["data_weights"], dtype=np.float32)
    db = np.asarray(inputs["data_biases"], dtype=np.float32)
    ew = np.asarray(inputs["edge_weights"], dtype=np.float32)
    eb = np.asarray(inputs["edge_biases"], dtype=np.float32)
    sew = np.asarray(inputs["score_embedding_weights"], dtype=np.float32)
    sdw = np.asarray(inputs["score_data_weights"], dtype=np.float32)
    sb = np.asarray(inputs["score_bias"], dtype=np.float32)

    scheds = [GraphSchedule(graphs[j], edges, pos) for j in range(G)]
    nc = _build_program(scheds)

    # ---- host-side data prep ----
    gv_rows = dv[data]                    # (N, VEC) gathered word vectors
    gv_pos = dv[data[pos]]
    dw_pad = np.zeros((VEC_PAD, D), np.float32)
    dw_pad[:VEC] = dw
    dw_in = dw_pad.reshape(3, 128, D)

    shared = {
        "dw": dw_in.astype(np.float16),
        "eb": eb.astype(np.float16),
        "ebT": np.ascontiguousarray(eb.T),
        "db": db.reshape(D, 1),
        "scw": np.ascontiguousarray(np.concatenate([sew, sdw], axis=1)),
        "sb": sb.reshape(1, 1),
    }

    in_maps = []
    for j, S in enumerate(scheds):
        gvT = np.zeros((VEC_PAD, NCOLS), np.float16)
        for n in range(N):
            gvT[:VEC, S.col[n]] = gv_rows[n]
        gvT[:VEC, POS_PURE_COL] = gv_pos
        wbuf = np.ascontiguousarray(
            ew[S.w_perm].transpose(1, 0, 2).reshape(D, E * D).astype(np.float16))
        m = dict(shared)
        m["gvT"] = gvT.reshape(3, 128, NCOLS)
        m["mult"] = S.mult.astype(np.float16)
        m["wbuf"] = wbuf
        in_maps.append(m)

    res = run_bass_kernel_spmd(nc, in_maps, core_ids=list(range(G)),
                               trace=bool(os.environ.get("BASS_TRACE")))
    LAST_RESULT = res
    out = np.stack([res.results[j]["scores"][0] for j in range(G)])
    return out.astype(np.float32)


# revision 16
# speedup vs baseline: 1.1640x; 1.1640x over previous
"""Trainium2 Bass kernel for nn_Net_49177375539428 (gnn_message_passing).

Strategy (see schedule builder below):
  - One core per candidate graph (8 graphs, 8 NeuronCores), single SPMD
    program with an 8-way switch on partition id; each branch is fully
    specialized to its graph's tree.
  - The (E,D) embedding matrices of the reference are row-constant except on
    the ancestor path of `pos`, so the computation decomposes into
      * a scalar chain: one vector x matrix transform per node (LDW + 1-col
        matmul accumulating straight into the parent's PSUM column,
        transposed layout [d, node]),
      * a branch at `pos` over all E edge matrices (1-col matmuls into a
        [d, e] PSUM tile, sharing weight loads with the chain),
      * ~log N full (D,D) matmuls along the pos->root path.
  - Edge weights are shipped once per core (8 MB) in first-use order and
    streamed through SBUF in chunks so DMA overlaps the PE work.
"""

import os
import numpy as np

import concourse.bass as bass
import concourse.mybir as mybir
import concourse.tile as tile
from concourse import bacc
from concourse.bass_utils import run_bass_kernel_spmd

N = 128          # nodes per graph
E = 128          # edge types
D = 128          # embedding dim
G = 8            # graphs / cores
VEC = 300        # word-vec dim
VEC_PAD = 384    # padded to 3x128
NCOLS = 132      # EMB columns: 128 nodes + pos_pure + pad to 4
POS_PURE_COL = 128
W_CHUNK_SLOTS = 32         # edge matrices per DMA chunk
N_W_CHUNKS = E // W_CHUNK_SLOTS

F32 = mybir.dt.float32
F16 = mybir.dt.float16

LAST_RESULT = None         # BassKernelResults of the most recent run


# ----------------------------------------------------------------------------
# Host-side schedule construction
# ----------------------------------------------------------------------------

class GraphSchedule:
    """Per-graph specialization: column assignment, wave structure, matmul
    schedule entries, and the per-core data (weight order, gvT, Mult)."""

    def __init__(self, g_row, edges, pos):
        parents = np.empty(N, np.int64)
        for i in range(N - 1):
            parents[i] = i + int(g_row[i])
        parents[N - 1] = -1
        children = [[] for _ in range(N)]
        for i in range(N - 1):
            children[parents[i]].append(i)
        internal = np.array([len(children[n]) > 0 for n in range(N)])
        depth = np.zeros(N, np.int64)
        for i in range(N - 2, -1, -1):
            depth[i] = depth[parents[i]] + 1
        maxdepth = int(depth.max())

        assert pos != N - 1, "pos == root not supported"
        path = []
        n = pos
        while n != N - 1:
            n = parents[n]
            path.append(n)
        pathset = set(path)

        # subtree of pos (incl. pos)
        sub = set()
        stack = [pos]
        while stack:
            n = stack.pop()
            sub.add(n)
            stack.extend(children[n])

        # Column assignment, level-major.  Within each level:
        #   [subtree-internal | other-internal | path nodes | leaves]
        col = np.full(N, -1, np.int64)
        self.sub_int_range = {}   # lvl -> (start, end)
        self.oth_int_range = {}   # lvl -> (start, end)
        off = 0
        lvl_nodes = [[] for _ in range(maxdepth + 1)]
        for n in range(N):
            lvl_nodes[depth[n]].append(n)
        for lvl in range(maxdepth + 1):
            nodes = lvl_nodes[lvl]
            sub_int = [n for n in nodes if n in sub and internal[n]]
            oth_int = [n for n in nodes
                       if internal[n] and n not in sub and n not in pathset]
            pth = [n for n in nodes if n in pathset]
            leaves = [n for n in nodes if not internal[n] and n not in pathset]
            self.sub_int_range[lvl] = (off, off + len(sub_int))
            for n in sub_int:
                col[n] = off
                off += 1
            self.oth_int_range[lvl] = (off, off + len(oth_int))
            for n in oth_int:
                col[n] = off
                off += 1
            for n in pth:
                col[n] = off
                off += 1
            for n in leaves:
                col[n] = off
                off += 1
        assert off == N

        self.parents, self.children = parents, children
        self.internal, self.depth, self.maxdepth = internal, depth, maxdepth
        self.path, self.pathset, self.sub = path, pathset, sub
        self.col = col
        self.pos = pos
        self.edges = edges
        self.path_idx = {a: k for k, a in enumerate(path)}
        # does path node have any chain (non-path, non-pos) children?
        self.path_has_chain = {
            a: any((c not in pathset) and c != pos for c in children[a])
            for a in path
        }

        self._build_entries()
        self._build_data_tables()

    def _build_entries(self):
        """Entries: (edge, [(src_col, psum_name, dst_col, start, stop)]).
        psum tiles: 'mini{lvl}', 'wave{lvl}', 'path', 'branch'."""
        edges, children, depth = self.edges, self.children, self.depth
        pos, sub, pathset = self.pos, self.sub, self.pathset
        col = self.col

        entries = []          # list of (edge_id, mm list)
        self.finalizes = []   # (after_entry_index, psum_name, psum_lo, psum_hi,
                              #  emb_lo, emb_hi)  -> EMB[lo:hi] = relu(psum+EMB)
        self.psum_sizes = {}

        # start/stop bookkeeping per (psum_name, dst_col)
        first_write = {}

        internal = self.internal
        pos_base = not internal[pos]

        def add_wave(kids_by_edge, psum_name, dst_of, branch_ok):
            """kids grouped per edge; appends entries (leaf-src edges first
            so they can overlap the previous wave's finalize)."""
            writer_cnt = {}
            for e, kids in kids_by_edge.items():
                for c in kids:
                    d = dst_of(c)
                    writer_cnt[d] = writer_cnt.get(d, 0) + 1
            seen_cnt = {}
            edge_order = sorted(kids_by_edge,
                                key=lambda e: (any(internal[c]
                                                   for c in kids_by_edge[e]), e))
            for e in edge_order:
                mms = []
                for c in kids_by_edge[e]:
                    d = dst_of(c)
                    seen_cnt[d] = seen_cnt.get(d, 0) + 1
                    key = (psum_name, d)
                    start = key not in first_write
                    first_write[key] = True
                    stop = seen_cnt[d] == writer_cnt[d]
                    mms.append((col[c], not internal[c],
                                psum_name, d, start, stop))
                if branch_ok and not self.branch_done[e]:
                    self.branch_done[e] = True
                    mms.append((col[pos], pos_base, 'branch', e, True, True))
                entries.append((e, mms))

        self.branch_done = [False] * E

        # --- mini waves: subtree of pos, deepest level first -----------------
        sub_lvls = sorted({int(depth[n]) for n in sub}, reverse=True)
        for lvl in sub_lvls:
            # children at this level whose parents are in sub at lvl-1
            kids = [n for n in sub if depth[n] == lvl and n != pos]
            if kids:
                by_edge = {}
                for c in kids:
                    by_edge.setdefault(int(edges[c]), []).append(c)
                add_wave(by_edge, f'mini{lvl}',
                         lambda c: col[self.parents[c]]
                         - self.sub_int_range[int(depth[c]) - 1][0],
                         branch_ok=False)
                lo, hi = self.sub_int_range[lvl - 1]
                self.psum_sizes[f'mini{lvl}'] = hi - lo
                self.finalizes.append((len(entries), f'mini{lvl}',
                                       0, hi - lo, lo, hi))
        # v_pos is now available (pos is leaf, or finalized by last mini wave)

        # --- main waves ------------------------------------------------------
        for lvl in range(self.maxdepth, 0, -1):
            kids = [n for n in range(N)
                    if depth[n] == lvl and n not in sub and n not in pathset]
            olo, ohi = self.oth_int_range[lvl - 1]
            if kids:
                by_edge = {}
                for c in kids:
                    by_edge.setdefault(int(edges[c]), []).append(c)

                def dst_of(c):
                    p = self.parents[c]
                    if p in pathset:
                        return ('path', self.path_idx[p])
                    return (f'wave{lvl - 1}', col[p] - olo)

                wcnt = {}
                for e, kids_e in by_edge.items():
                    for c in kids_e:
                        tgt = dst_of(c)
                        wcnt[tgt] = wcnt.get(tgt, 0) + 1
                seen = {}
                edge_order = sorted(by_edge,
                                    key=lambda e: (any(internal[c]
                                                       for c in by_edge[e]), e))
                for e in edge_order:
                    mms = []
                    for c in by_edge[e]:
                        name, d = dst_of(c)
                        tgt = (name, d)
                        seen[tgt] = seen.get(tgt, 0) + 1
                        start = tgt not in first_write
                        first_write[tgt] = True
                        stop = seen[tgt] == wcnt[tgt]
                        mms.append((col[c], not internal[c], name, d, start, stop))
                    if not self.branch_done[e]:
                        self.branch_done[e] = True
                        mms.append((col[pos], pos_base, 'branch', e, True, True))
                    entries.append((e, mms))
            if ohi > olo:
                self.psum_sizes[f'wave{lvl - 1}'] = ohi - olo
                self.finalizes.append((len(entries), f'wave{lvl - 1}',
                                       0, ohi - olo, olo, ohi))

        # --- leftover branch edges ------------------------------------------
        for e in range(E):
            if not self.branch_done[e]:
                self.branch_done[e] = True
                entries.append((e, [(self.col[pos], pos_base,
                                     'branch', e, True, True)]))

        self.psum_sizes['branch'] = E
        self.psum_sizes['path'] = max(1, len(self.path))

        # PSUM start/stop semantics: start=True lazily zeroes the ENTIRE
        # 2KB bank (pending-zero), after which the per-byte pending flag
        # makes fresh columns overwrite and touched columns accumulate.
        # So: start only on the very first matmul into each tile, stop on
        # the last.  (Per-column start flags would wipe sibling columns.)
        totals = {}
        for _, mms in entries:
            for (_, _, pname, _, _, _) in mms:
                totals[pname] = totals.get(pname, 0) + 1
        seen = {}
        fixed = []
        for e, mms in entries:
            new_mms = []
            for (src, sbase, pname, dst, _, _) in mms:
                k = seen.get(pname, 0)
                seen[pname] = k + 1
                new_mms.append((src, sbase, pname, dst,
                                k == 0, k + 1 == totals[pname]))
            fixed.append((e, new_mms))
        self.entries = fixed

        # Wbuf slot order = first use
        slot_of = {}
        for e, _ in entries:
            if e not in slot_of:
                slot_of[e] = len(slot_of)
        assert len(slot_of) == E
        self.slot_of = slot_of

    def _build_data_tables(self):
        """Per-core numpy inputs: Wbuf (first-use order), Mult, perm for gvT."""
        perm = np.empty(E, np.int64)     # slot -> edge id
        for e, s in self.slot_of.items():
            perm[s] = e
        self.w_perm = perm

        # multiplicity matrix: Mult[e, col(p)] = # chain children of p with edge e
        mult = np.zeros((E, NCOLS), np.float32)
        for p in range(N):
            for c in self.children[p]:
                if c in self.pathset or c == self.pos:
                    continue
                mult[int(self.edges[c]), self.col[p]] += 1.0
        self.mult = mult


# ----------------------------------------------------------------------------
# Bass program
# ----------------------------------------------------------------------------

def _build_program(scheds):
    nc = bacc.Bacc("TRN2", target_bir_lowering=False, debug=False, num_devices=G)

    # shared inputs
    t_dw = nc.declare_dram_parameter("dw", [3, 128, D], F16, isOutput=False)
    t_eb = nc.declare_dram_parameter("eb", [E, D], F16, isOutput=False)
    t_ebT = nc.declare_dram_parameter("ebT", [D, E], F32, isOutput=False)
    t_db = nc.declare_dram_parameter("db", [D, 1], F32, isOutput=False)
    t_scw = nc.declare_dram_parameter("scw", [D, 2], F32, isOutput=False)
    t_sb = nc.declare_dram_parameter("sb", [1, 1], F32, isOutput=False)
    # per-core inputs
    t_gvT = nc.declare_dram_parameter("gvT", [3, 128, NCOLS], F16, isOutput=False)
    t_mult = nc.declare_dram_parameter("mult", [E, NCOLS], F16, isOutput=False)
    t_w = nc.declare_dram_parameter("wbuf", [D, E * D], F16, isOutput=False)
    t_out = nc.declare_dram_parameter("scores", [1, E], F32, isOutput=True)

    with tile.TileContext(nc) as tc:
        with (
            tc.tile_pool(name="wpool", bufs=1) as wpool,
            tc.tile_pool(name="sbuf", bufs=1) as pool,
            tc.tile_pool(name="ppool", bufs=2, space="PSUM") as ppool,
            tc.tile_pool(name="ppool_fix", bufs=1, space="PSUM") as ppool_fix,
        ):
            # All DMA loads are identical instructions across graphs (per-core
            # content differs via in_maps) — issue them before the Switch so
            # transfers stream from t=0, overlapping dispatch + compute.
            # W chunks first, all on the sync HWDGE: triggers land at t~0
            # and the hardware DGE streams the 4.2MB autonomously.
            wb = t_w.ap().rearrange("p (c s) -> p c s", c=N_W_CHUNKS)
            w_chunks = []
            for c in range(N_W_CHUNKS):
                wt = wpool.tile([D, W_CHUNK_SLOTS * D], F16, tag=f"w{c}",
                                name=f"w{c}")
                nc.sync.dma_start(wt[:], wb[:, c, :])
                w_chunks.append(wt)
            sb_tiles = {}
            sb_tiles['dw'] = pool.tile([128, 3, D], F16, tag="dw", name="dw")
            nc.scalar.dma_start(sb_tiles['dw'][:], t_dw.ap().rearrange("c p d -> p c d"))
            sb_tiles['gv'] = pool.tile([128, 3, NCOLS], F16, tag="gv", name="gv")
            nc.scalar.dma_start(sb_tiles['gv'][:], t_gvT.ap().rearrange("c p d -> p c d"))
            sb_tiles['mult'] = pool.tile([E, NCOLS], F16, tag="mult", name="mult")
            nc.scalar.dma_start(sb_tiles['mult'][:], t_mult[:])
            sb_tiles['eb'] = pool.tile([E, D], F16, tag="eb", name="eb")
            nc.scalar.dma_start(sb_tiles['eb'][:], t_eb[:])
            sb_tiles['ebT'] = pool.tile([D, E], F32, tag="ebT", name="ebT")
            nc.scalar.dma_start(sb_tiles['ebT'][:], t_ebT[:])
            sb_tiles['db'] = pool.tile([D, 1], F32, tag="db", name="db")
            nc.scalar.dma_start(sb_tiles['db'][:], t_db[:])
            sb_tiles['scw'] = pool.tile([D, 2], F32, tag="scw", name="scw")
            nc.scalar.dma_start(sb_tiles['scw'][:], t_scw[:])
            sb_tiles['sb'] = pool.tile([1, 1], F32, tag="sb", name="sb")
            nc.scalar.dma_start(sb_tiles['sb'][:], t_sb[:])
            pid = nc.partition_id()
            for j in tc.Switch(pid, G):
                _emit_graph(nc, scheds[j], pool, ppool, ppool_fix,
                            sb_tiles, w_chunks, t_out)
    nc.finalize()
    return nc


def _emit_graph(nc, S, pool, ppool, ppool_fix, sb_tiles, w_chunks, t_out):
    Relu = mybir.ActivationFunctionType.Relu
    ADD = mybir.AluOpType.add
    dw_sb = sb_tiles['dw']
    eb_sb = sb_tiles['eb']
    ebT_sb = sb_tiles['ebT']
    db_sb = sb_tiles['db']
    scw_sb = sb_tiles['scw']
    sb_sb = sb_tiles['sb']
    gv_sb = sb_tiles['gv']
    mult_sb = sb_tiles['mult']

    def w_ap(edge):
        s = S.slot_of[edge]
        return w_chunks[s // W_CHUNK_SLOTS][
            :, (s % W_CHUNK_SLOTS) * D:(s % W_CHUNK_SLOTS + 1) * D]

    # ---- EMB = baseB (base + bias-sums), then finalized in-place ----
    ps_base = ppool_fix.tile([128, NCOLS], F32, tag="ps_base")
    for k in range(3):
        nc.tensor.matmul(ps_base[:], dw_sb[:, k, :], gv_sb[:, k, :],
                         start=(k == 0), stop=False)
    nc.tensor.matmul(ps_base[:], eb_sb[:], mult_sb[:], start=False, stop=True)
    emb = pool.tile([128, NCOLS], F32, tag="emb")
    nc.vector.tensor_scalar(emb[:], ps_base[:], db_sb[:, 0:1], None, ADD)
    emb16b = pool.tile([128, NCOLS], F16, tag="emb16b")   # static: leaves etc.
    nc.vector.tensor_copy(emb16b[:], emb[:])
    emb16 = pool.tile([128, NCOLS], F16, tag="emb16")     # finalize targets

    # ---- psum tiles for waves / branch / path ----
    ps = {}
    ps['branch'] = ppool_fix.tile([128, E], F32, tag="ps_branch", name="ps_branch")
    ps['path'] = ppool_fix.tile([128, S.psum_sizes['path']], F32, tag="ps_path", name="ps_path")
    for name, sz in S.psum_sizes.items():
        if name in ('branch', 'path'):
            continue
        ps[name] = ppool.tile([128, sz], F32, tag="ps_wave", name=f"ps_{name}")

    # ---- chain + branch matmuls with interleaved finalizes ----
    fin = list(S.finalizes)
    fi = 0
    for idx, (e, mms) in enumerate(S.entries):
        while fi < len(fin) and fin[fi][0] == idx:
            _finalize(nc, emb, emb16, ps, fin[fi])
            fi += 1
        wap = w_ap(e)
        for (src, sbase, pname, dst, start, stop) in mms:
            mv = emb16b if sbase else emb16
            nc.tensor.matmul(ps[pname][:, dst:dst + 1], wap,
                             mv[:, src:src + 1], start=start, stop=stop)
    while fi < len(fin):
        _finalize(nc, emb, emb16, ps, fin[fi])
        fi += 1

    # ---- path walk ----
    # step k at ancestor a_k:  M_k = relu(prev_transform + b_{e_{k-1}} + u_k)
    # with u_k = baseB[a_k] (+ chain psum).  Bias columns are precomputed so
    # each step is ONE fused DVE op (add per-partition bias, max 0) + one MM.
    plen = len(S.path)
    biases = pool.tile([128, max(plen, 1)], F32, tag="biases")
    for k, a in enumerate(S.path):
        acol = int(S.col[a])
        eprev = None if k == 0 else int(S.edges[S.path[k - 1]])
        if S.path_has_chain[a]:
            nc.vector.tensor_tensor(
                biases[:, k:k + 1], ps['path'][:, k:k + 1],
                emb[:, acol:acol + 1], ADD)
            if eprev is not None:
                nc.vector.tensor_tensor(
                    biases[:, k:k + 1], biases[:, k:k + 1],
                    ebT_sb[:, eprev:eprev + 1], ADD)
        elif eprev is not None:
            nc.vector.tensor_tensor(
                biases[:, k:k + 1], emb[:, acol:acol + 1],
                ebT_sb[:, eprev:eprev + 1], ADD)
        else:
            nc.vector.tensor_copy(biases[:, k:k + 1], emb[:, acol:acol + 1])

    mcur = pool.tile([128, E], F32, tag="mcur")
    mnext = pool.tile([128, E], F16, tag="mnext")
    cur_ps = ps['branch']
    for k, a in enumerate(S.path):
        if k == 0:
            # branch contribution carries full per-column edge biases
            nc.vector.tensor_tensor(mcur[:], cur_ps[:], ebT_sb[:], ADD)
            nc.vector.tensor_scalar(mnext[:], mcur[:], biases[:, 0:1], 0.0,
                                    ADD, mybir.AluOpType.max)
        else:
            nc.vector.tensor_scalar(mnext[:], cur_ps[:], biases[:, k:k + 1],
                                    0.0, ADD, mybir.AluOpType.max)
        ea = int(S.edges[a])
        ps_step = ppool_fix.tile([128, E], F32, tag="ps_step")
        nc.tensor.matmul(ps_step[:], w_ap(ea), mnext[:], start=True, stop=True)
        cur_ps = ps_step

    # res^T = ps_step + b_eroot  [d, e]
    eroot = int(S.edges[N - 1])
    nc.vector.tensor_scalar(mcur[:], cur_ps[:],
                            ebT_sb[:, eroot:eroot + 1], None, ADD)
    ps_sc = ppool_fix.tile([1, E + 4], F32, tag="ps_sc")
    nc.tensor.matmul(ps_sc[:, 0:E], scw_sb[:, 0:1], mcur[:],
                     start=True, stop=False)
    nc.tensor.matmul(ps_sc[:, E:E + 1], scw_sb[:, 1:2],
                     emb[:, POS_PURE_COL:POS_PURE_COL + 1], start=False, stop=True)
    dsc = pool.tile([1, 1], F32, tag="dsc")
    nc.vector.tensor_tensor(dsc[:], ps_sc[:, E:E + 1], sb_sb[:], ADD)
    srow = pool.tile([1, E], F32, tag="srow")
    nc.vector.tensor_scalar(srow[:], ps_sc[:, 0:E], dsc[:], None, ADD)
    nc.sync.dma_start(t_out[:], srow[:])


def _finalize(nc, emb, emb16, ps, f):
    _, name, plo, phi, elo, ehi = f
    if phi <= plo:
        return
    nc.vector.tensor_tensor(emb[:, elo:ehi], ps[name][:, plo:phi],
                            emb[:, elo:ehi], mybir.AluOpType.add)
    nc.vector.tensor_scalar(emb16[:, elo:ehi], emb[:, elo:ehi],
                            0.0, None, mybir.AluOpType.max)


# ----------------------------------------------------------------------------
# Host entry point
# ----------------------------------------------------------------------------

def kernel(**inputs):
    global LAST_RESULT
    data = np.asarray(inputs["data"])
    graphs = np.asarray(inputs["graphs"])
    edges = np.asarray(inputs["edges"])
    pos = int(np.asarray(inputs["pos"]))
    dv = np.asarray(inputs["data_vecs"], dtype=np.float32)
    dw = np.asarray(inputs["data_weights"], dtype=np.float32)
    db = np.asarray(inputs["data_biases"], dtype=np.float32)
    ew = np.asarray(inputs["edge_weights"], dtype=np.float32)
    eb = np.asarray(inputs["edge_biases"], dtype=np.float32)
    sew = np.asarray(inputs["score_embedding_weights"], dtype=np.float32)
    sdw = np.asarray(inputs["score_data_weights"], dtype=np.float32)
    sb = np.asarray(inputs["score_bias"], dtype=np.float32)

    scheds = [GraphSchedule(graphs[j], edges, pos) for j in range(G)]
    nc = _build_program(scheds)

    # ---- host-side data prep ----
    gv_rows = dv[data]                    # (N, VEC) gathered word vectors
    gv_pos = dv[data[pos]]
    dw_pad = np.zeros((VEC_PAD, D), np.float32)
    dw_pad[:VEC] = dw
    dw_in = dw_pad.reshape(3, 128, D)

    shared = {
        "dw": dw_in.astype(np.float16),
        "eb": eb.astype(np.float16),
        "ebT": np.ascontiguousarray(eb.T),
        "db": db.reshape(D, 1),
        "scw": np.ascontiguousarray(np.concatenate([sew, sdw], axis=1)),
        "sb": sb.reshape(1, 1),
    }

    in_maps = []
    for j, S in enumerate(scheds):
        gvT = np.zeros((VEC_PAD, NCOLS), np.float16)
        for n in range(N):
            gvT[:VEC, S.col[n]] = gv_rows[n]
        gvT[:VEC, POS_PURE_COL] = gv_pos
        wbuf = np.ascontiguousarray(
            ew[S.w_perm].transpose(1, 0, 2).reshape(D, E * D).astype(np.float16))
        m = dict(shared)
        m["gvT"] = gvT.reshape(3, 128, NCOLS)
        m["mult"] = S.mult.astype(np.float16)
        m["wbuf"] = wbuf
        in_maps.append(m)

    res = run_bass_kernel_spmd(nc, in_maps, core_ids=list(range(G)),
                               trace=bool(os.environ.get("BASS_TRACE")))
    LAST_RESULT = res
    out = np.stack([res.results[j]["scores"][0] for j in range(G)])
    return out.astype(np.float32)


# revision 17
# speedup vs baseline: 1.1786x; 1.0126x over previous
"""Trainium2 Bass kernel for nn_Net_49177375539428 (gnn_message_passing).

Strategy (see schedule builder below):
  - One core per candidate graph (8 graphs, 8 NeuronCores), single SPMD
    program with an 8-way switch on partition id; each branch is fully
    specialized to its graph's tree.
  - The (E,D) embedding matrices of the reference are row-constant except on
    the ancestor path of `pos`, so the computation decomposes into
      * a scalar chain: one vector x matrix transform per node (LDW + 1-col
        matmul accumulating straight into the parent's PSUM column,
        transposed layout [d, node]),
      * a branch at `pos` over all E edge matrices (1-col matmuls into a
        [d, e] PSUM tile, sharing weight loads with the chain),
      * ~log N full (D,D) matmuls along the pos->root path.
  - Edge weights are shipped once per core (8 MB) in first-use order and
    streamed through SBUF in chunks so DMA overlaps the PE work.
"""

import os
import numpy as np

import concourse.bass as bass
import concourse.mybir as mybir
import concourse.tile as tile
from concourse import bacc
from concourse.bass_utils import run_bass_kernel_spmd

N = 128          # nodes per graph
E = 128          # edge types
D = 128          # embedding dim
G = 8            # graphs / cores
VEC = 300        # word-vec dim
VEC_PAD = 384    # padded to 3x128
NCOLS = 132      # EMB columns: 128 nodes + pos_pure + pad to 4
POS_PURE_COL = 128
W_CHUNK_SLOTS = 32         # edge matrices per DMA chunk
N_W_CHUNKS = E // W_CHUNK_SLOTS

F32 = mybir.dt.float32
F16 = mybir.dt.float16

LAST_RESULT = None         # BassKernelResults of the most recent run


# ----------------------------------------------------------------------------
# Host-side schedule construction
# ----------------------------------------------------------------------------

class GraphSchedule:
    """Per-graph specialization: column assignment, wave structure, matmul
    schedule entries, and the per-core data (weight order, gvT, Mult)."""

    def __init__(self, g_row, edges, pos):
        parents = np.empty(N, np.int64)
        for i in range(N - 1):
            parents[i] = i + int(g_row[i])
        parents[N - 1] = -1
        children = [[] for _ in range(N)]
        for i in range(N - 1):
            children[parents[i]].append(i)
        internal = np.array([len(children[n]) > 0 for n in range(N)])
        depth = np.zeros(N, np.int64)
        for i in range(N - 2, -1, -1):
            depth[i] = depth[parents[i]] + 1
        maxdepth = int(depth.max())

        assert pos != N - 1, "pos == root not supported"
        path = []
        n = pos
        while n != N - 1:
            n = parents[n]
            path.append(n)
        pathset = set(path)

        # subtree of pos (incl. pos)
        sub = set()
        stack = [pos]
        while stack:
            n = stack.pop()
            sub.add(n)
            stack.extend(children[n])

        # Column assignment, level-major.  Within each level:
        #   [subtree-internal | other-internal | path nodes | leaves]
        col = np.full(N, -1, np.int64)
        self.sub_int_range = {}   # lvl -> (start, end)
        self.oth_int_range = {}   # lvl -> (start, end)
        off = 0
        lvl_nodes = [[] for _ in range(maxdepth + 1)]
        for n in range(N):
            lvl_nodes[depth[n]].append(n)
        for lvl in range(maxdepth + 1):
            nodes = lvl_nodes[lvl]
            sub_int = [n for n in nodes if n in sub and internal[n]]
            oth_int = [n for n in nodes
                       if internal[n] and n not in sub and n not in pathset]
            pth = [n for n in nodes if n in pathset]
            leaves = [n for n in nodes if not internal[n] and n not in pathset]
            self.sub_int_range[lvl] = (off, off + len(sub_int))
            for n in sub_int:
                col[n] = off
                off += 1
            self.oth_int_range[lvl] = (off, off + len(oth_int))
            for n in oth_int:
                col[n] = off
                off += 1
            for n in pth:
                col[n] = off
                off += 1
            for n in leaves:
                col[n] = off
                off += 1
        assert off == N

        self.parents, self.children = parents, children
        self.internal, self.depth, self.maxdepth = internal, depth, maxdepth
        self.path, self.pathset, self.sub = path, pathset, sub
        self.col = col
        self.pos = pos
        self.edges = edges
        self.path_idx = {a: k for k, a in enumerate(path)}
        # does path node have any chain (non-path, non-pos) children?
        self.path_has_chain = {
            a: any((c not in pathset) and c != pos for c in children[a])
            for a in path
        }

        self._build_entries()
        self._build_data_tables()

    def _build_entries(self):
        """Entries: (edge, [(src_col, psum_name, dst_col, start, stop)]).
        psum tiles: 'mini{lvl}', 'wave{lvl}', 'path', 'branch'."""
        edges, children, depth = self.edges, self.children, self.depth
        pos, sub, pathset = self.pos, self.sub, self.pathset
        col = self.col

        entries = []          # list of (edge_id, mm list)
        self.finalizes = []   # (after_entry_index, psum_name, psum_lo, psum_hi,
                              #  emb_lo, emb_hi)  -> EMB[lo:hi] = relu(psum+EMB)
        self.psum_sizes = {}

        # start/stop bookkeeping per (psum_name, dst_col)
        first_write = {}

        internal = self.internal
        pos_base = not internal[pos]

        def add_wave(kids_by_edge, psum_name, dst_of, branch_ok):
            """kids grouped per edge; appends entries (leaf-src edges first
            so they can overlap the previous wave's finalize)."""
            writer_cnt = {}
            for e, kids in kids_by_edge.items():
                for c in kids:
                    d = dst_of(c)
                    writer_cnt[d] = writer_cnt.get(d, 0) + 1
            seen_cnt = {}
            edge_order = sorted(kids_by_edge,
                                key=lambda e: (any(internal[c]
                                                   for c in kids_by_edge[e]), e))
            for e in edge_order:
                mms = []
                for c in kids_by_edge[e]:
                    d = dst_of(c)
                    seen_cnt[d] = seen_cnt.get(d, 0) + 1
                    key = (psum_name, d)
                    start = key not in first_write
                    first_write[key] = True
                    stop = seen_cnt[d] == writer_cnt[d]
                    mms.append((col[c], not internal[c],
                                psum_name, d, start, stop))
                if branch_ok and not self.branch_done[e]:
                    self.branch_done[e] = True
                    mms.append((col[pos], pos_base, 'branch', e, True, True))
                entries.append(('W', e, mms))

        self.branch_done = [False] * E

        # --- mini waves: subtree of pos, deepest level first -----------------
        sub_lvls = sorted({int(depth[n]) for n in sub}, reverse=True)
        for lvl in sub_lvls:
            # children at this level whose parents are in sub at lvl-1
            kids = [n for n in sub if depth[n] == lvl and n != pos]
            if kids:
                slo, shi = self.sub_int_range[lvl - 1]
                entries.append(('B', f'mini{lvl}', slo, shi))
                by_edge = {}
                for c in kids:
                    by_edge.setdefault(int(edges[c]), []).append(c)
                add_wave(by_edge, f'mini{lvl}',
                         lambda c: col[self.parents[c]]
                         - self.sub_int_range[int(depth[c]) - 1][0],
                         branch_ok=False)
                lo, hi = self.sub_int_range[lvl - 1]
                self.psum_sizes[f'mini{lvl}'] = hi - lo
                self.finalizes.append((len(entries), f'mini{lvl}',
                                       0, hi - lo, lo, hi))
        # v_pos is now available (pos is leaf, or finalized by last mini wave)

        # --- main waves ------------------------------------------------------
        for lvl in range(self.maxdepth, 0, -1):
            kids = [n for n in range(N)
                    if depth[n] == lvl and n not in sub and n not in pathset]
            olo, ohi = self.oth_int_range[lvl - 1]
            if ohi > olo:
                entries.append(('B', f'wave{lvl - 1}', olo, ohi))
            if kids:
                by_edge = {}
                for c in kids:
                    by_edge.setdefault(int(edges[c]), []).append(c)

                def dst_of(c):
                    p = self.parents[c]
                    if p in pathset:
                        return ('path', self.path_idx[p])
                    return (f'wave{lvl - 1}', col[p] - olo)

                wcnt = {}
                for e, kids_e in by_edge.items():
                    for c in kids_e:
                        tgt = dst_of(c)
                        wcnt[tgt] = wcnt.get(tgt, 0) + 1
                seen = {}
                edge_order = sorted(by_edge,
                                    key=lambda e: (any(internal[c]
                                                       for c in by_edge[e]), e))
                for e in edge_order:
                    mms = []
                    for c in by_edge[e]:
                        name, d = dst_of(c)
                        tgt = (name, d)
                        seen[tgt] = seen.get(tgt, 0) + 1
                        start = tgt not in first_write
                        first_write[tgt] = True
                        stop = seen[tgt] == wcnt[tgt]
                        mms.append((col[c], not internal[c], name, d, start, stop))
                    if not self.branch_done[e]:
                        self.branch_done[e] = True
                        mms.append((col[pos], pos_base, 'branch', e, True, True))
                    entries.append(('W', e, mms))
            if ohi > olo:
                self.psum_sizes[f'wave{lvl - 1}'] = ohi - olo
                self.finalizes.append((len(entries), f'wave{lvl - 1}',
                                       0, ohi - olo, olo, ohi))

        # --- leftover branch edges ------------------------------------------
        for e in range(E):
            if not self.branch_done[e]:
                self.branch_done[e] = True
                entries.append(('W', e, [(self.col[pos], pos_base,
                                          'branch', e, True, True)]))

        self.psum_sizes['branch'] = E
        self.psum_sizes['path'] = max(1, len(self.path))

        # PSUM start/stop semantics: start=True lazily zeroes the ENTIRE
        # 2KB bank (pending-zero), after which the per-byte pending flag
        # makes fresh columns overwrite and touched columns accumulate.
        # So: start only on the very first matmul into each tile, stop on
        # the last.  (Per-column start flags would wipe sibling columns.)
        totals = {}
        for ent in entries:
            if ent[0] == 'B':
                totals[ent[1]] = totals.get(ent[1], 0) + 4
            else:
                for (_, _, pname, _, _, _) in ent[2]:
                    totals[pname] = totals.get(pname, 0) + 1
        seen = {}
        fixed = []
        for ent in entries:
            if ent[0] == 'B':
                _, pname, lo, hi = ent
                k = seen.get(pname, 0)
                seen[pname] = k + 4
                fixed.append(('B', pname, lo, hi, k == 0,
                              k + 4 == totals[pname]))
                continue
            _, e, mms = ent
            new_mms = []
            for (src, sbase, pname, dst, _, _) in mms:
                k = seen.get(pname, 0)
                seen[pname] = k + 1
                new_mms.append((src, sbase, pname, dst,
                                k == 0, k + 1 == totals[pname]))
            fixed.append(('W', e, new_mms))
        self.entries = fixed

        # Wbuf slot order = first use
        slot_of = {}
        for ent in entries:
            if ent[0] != 'W':
                continue
            e = ent[1]
            if e not in slot_of:
                slot_of[e] = len(slot_of)
        assert len(slot_of) == E
        self.slot_of = slot_of

    def _build_data_tables(self):
        """Per-core numpy inputs: Wbuf (first-use order), Mult, perm for gvT."""
        perm = np.empty(E, np.int64)     # slot -> edge id
        for e, s in self.slot_of.items():
            perm[s] = e
        self.w_perm = perm

        # multiplicity matrix: Mult[e, col(p)] = # chain children of p with edge e
        mult = np.zeros((E, NCOLS), np.float32)
        for p in range(N):
            for c in self.children[p]:
                if c in self.pathset or c == self.pos:
                    continue
                mult[int(self.edges[c]), self.col[p]] += 1.0
        self.mult = mult


# ----------------------------------------------------------------------------
# Bass program
# ----------------------------------------------------------------------------

def _build_program(scheds):
    nc = bacc.Bacc("TRN2", target_bir_lowering=False, debug=False, num_devices=G)

    # shared inputs
    t_dw = nc.declare_dram_parameter("dw", [3, 128, D], F16, isOutput=False)
    t_eb = nc.declare_dram_parameter("eb", [E, D], F16, isOutput=False)
    t_ebT = nc.declare_dram_parameter("ebT", [D, E], F32, isOutput=False)
    t_db = nc.declare_dram_parameter("db", [D, 1], F32, isOutput=False)
    t_scw = nc.declare_dram_parameter("scw", [D, 2], F32, isOutput=False)
    t_sb = nc.declare_dram_parameter("sb", [1, 1], F32, isOutput=False)
    # per-core inputs
    t_gvT = nc.declare_dram_parameter("gvT", [3, 128, NCOLS], F16, isOutput=False)
    t_mult = nc.declare_dram_parameter("mult", [E, NCOLS], F16, isOutput=False)
    t_w = nc.declare_dram_parameter("wbuf", [D, E * D], F16, isOutput=False)
    t_out = nc.declare_dram_parameter("scores", [1, E], F32, isOutput=True)

    with tile.TileContext(nc) as tc:
        with (
            tc.tile_pool(name="wpool", bufs=1) as wpool,
            tc.tile_pool(name="sbuf", bufs=1) as pool,
            tc.tile_pool(name="ppool", bufs=2, space="PSUM") as ppool,
            tc.tile_pool(name="ppool_fix", bufs=1, space="PSUM") as ppool_fix,
        ):
            # All DMA loads are identical instructions across graphs (per-core
            # content differs via in_maps) — issue them before the Switch so
            # transfers stream from t=0, overlapping dispatch + compute.
            # W chunks first, all on the sync HWDGE: triggers land at t~0
            # and the hardware DGE streams the 4.2MB autonomously.
            wb = t_w.ap().rearrange("p (c s) -> p c s", c=N_W_CHUNKS)
            w_chunks = []
            for c in range(N_W_CHUNKS):
                wt = wpool.tile([D, W_CHUNK_SLOTS * D], F16, tag=f"w{c}",
                                name=f"w{c}")
                nc.sync.dma_start(wt[:], wb[:, c, :])
                w_chunks.append(wt)
            sb_tiles = {}
            sb_tiles['dw'] = pool.tile([128, 3, D], F16, tag="dw", name="dw")
            nc.scalar.dma_start(sb_tiles['dw'][:], t_dw.ap().rearrange("c p d -> p c d"))
            sb_tiles['gv'] = pool.tile([128, 3, NCOLS], F16, tag="gv", name="gv")
            nc.scalar.dma_start(sb_tiles['gv'][:], t_gvT.ap().rearrange("c p d -> p c d"))
            sb_tiles['mult'] = pool.tile([E, NCOLS], F16, tag="mult", name="mult")
            nc.scalar.dma_start(sb_tiles['mult'][:], t_mult[:])
            sb_tiles['eb'] = pool.tile([E, D], F16, tag="eb", name="eb")
            nc.scalar.dma_start(sb_tiles['eb'][:], t_eb[:])
            sb_tiles['ebT'] = pool.tile([D, E], F32, tag="ebT", name="ebT")
            nc.scalar.dma_start(sb_tiles['ebT'][:], t_ebT[:])
            sb_tiles['db'] = pool.tile([D, 1], F32, tag="db", name="db")
            nc.scalar.dma_start(sb_tiles['db'][:], t_db[:])
            sb_tiles['scw'] = pool.tile([D, 2], F32, tag="scw", name="scw")
            nc.scalar.dma_start(sb_tiles['scw'][:], t_scw[:])
            sb_tiles['sb'] = pool.tile([1, 1], F32, tag="sb", name="sb")
            nc.scalar.dma_start(sb_tiles['sb'][:], t_sb[:])
            pid = nc.partition_id()
            for j in tc.Switch(pid, G):
                _emit_graph(nc, scheds[j], pool, ppool, ppool_fix,
                            sb_tiles, w_chunks, t_out)
    nc.finalize()
    return nc


def _emit_graph(nc, S, pool, ppool, ppool_fix, sb_tiles, w_chunks, t_out):
    Relu = mybir.ActivationFunctionType.Relu
    ADD = mybir.AluOpType.add
    dw_sb = sb_tiles['dw']
    eb_sb = sb_tiles['eb']
    ebT_sb = sb_tiles['ebT']
    db_sb = sb_tiles['db']
    scw_sb = sb_tiles['scw']
    sb_sb = sb_tiles['sb']
    gv_sb = sb_tiles['gv']
    mult_sb = sb_tiles['mult']

    def w_ap(edge):
        s = S.slot_of[edge]
        return w_chunks[s // W_CHUNK_SLOTS][
            :, (s % W_CHUNK_SLOTS) * D:(s % W_CHUNK_SLOTS + 1) * D]

    # ---- EMB = baseB (base + bias-sums), then finalized in-place ----
    ps_base = ppool_fix.tile([128, NCOLS], F32, tag="ps_base")
    for k in range(3):
        nc.tensor.matmul(ps_base[:], dw_sb[:, k, :], gv_sb[:, k, :],
                         start=(k == 0), stop=False)
    nc.tensor.matmul(ps_base[:], eb_sb[:], mult_sb[:], start=False, stop=True)
    emb = pool.tile([128, NCOLS], F32, tag="emb")
    nc.vector.tensor_copy(emb[:], ps_base[:])
    emb16b = pool.tile([128, NCOLS], F16, tag="emb16b")   # static: leaves etc.
    nc.vector.tensor_copy(emb16b[:], emb[:])
    emb16 = pool.tile([128, NCOLS], F16, tag="emb16")     # finalize targets

    # ---- psum tiles for waves / branch / path ----
    ps = {}
    ps['branch'] = ppool_fix.tile([128, E], F32, tag="ps_branch", name="ps_branch")
    ps['path'] = ppool_fix.tile([128, S.psum_sizes['path']], F32, tag="ps_path", name="ps_path")
    for name, sz in S.psum_sizes.items():
        if name in ('branch', 'path'):
            continue
        ps[name] = ppool.tile([128, sz], F32, tag="ps_wave", name=f"ps_{name}")

    # ---- chain + branch matmuls with interleaved finalizes ----
    fin = list(S.finalizes)
    fi = 0
    for idx, ent in enumerate(S.entries):
        while fi < len(fin) and fin[fi][0] == idx:
            _finalize(nc, emb16, ps, fin[fi])
            fi += 1
        if ent[0] == 'B':
            _, pname, lo, hi, start, stop = ent
            pt = ps[pname]
            w = hi - lo
            for k in range(3):
                nc.tensor.matmul(pt[:, 0:w], dw_sb[:, k, :],
                                 gv_sb[:, k, lo:hi],
                                 start=(start and k == 0), stop=False)
            nc.tensor.matmul(pt[:, 0:w], eb_sb[:], mult_sb[:, lo:hi],
                             start=False, stop=stop)
            continue
        _, e, mms = ent
        wap = w_ap(e)
        for (src, sbase, pname, dst, start, stop) in mms:
            mv = emb16b if sbase else emb16
            nc.tensor.matmul(ps[pname][:, dst:dst + 1], wap,
                             mv[:, src:src + 1], start=start, stop=stop)
    while fi < len(fin):
        _finalize(nc, emb16, ps, fin[fi])
        fi += 1

    # ---- path walk ----
    # step k at ancestor a_k:  M_k = relu(prev_transform + b_{e_{k-1}} + u_k)
    # with u_k = baseB[a_k] (+ chain psum).  Bias columns are precomputed so
    # each step is ONE fused DVE op (add per-partition bias, max 0) + one MM.
    plen = len(S.path)
    biases = pool.tile([128, max(plen, 1)], F32, tag="biases")
    for k, a in enumerate(S.path):
        acol = int(S.col[a])
        eprev = None if k == 0 else int(S.edges[S.path[k - 1]])
        if S.path_has_chain[a]:
            nc.vector.tensor_tensor(
                biases[:, k:k + 1], ps['path'][:, k:k + 1],
                emb[:, acol:acol + 1], ADD)
            if eprev is not None:
                nc.vector.tensor_tensor(
                    biases[:, k:k + 1], biases[:, k:k + 1],
                    ebT_sb[:, eprev:eprev + 1], ADD)
        elif eprev is not None:
            nc.vector.tensor_tensor(
                biases[:, k:k + 1], emb[:, acol:acol + 1],
                ebT_sb[:, eprev:eprev + 1], ADD)
        else:
            nc.vector.tensor_copy(biases[:, k:k + 1], emb[:, acol:acol + 1])

    mcur = pool.tile([128, E], F32, tag="mcur")
    mnext = pool.tile([128, E], F16, tag="mnext")
    cur_ps = ps['branch']
    for k, a in enumerate(S.path):
        if k == 0:
            # branch contribution carries full per-column edge biases
            nc.vector.tensor_tensor(mcur[:], cur_ps[:], ebT_sb[:], ADD)
            nc.vector.tensor_scalar(mnext[:], mcur[:], biases[:, 0:1], 0.0,
                                    ADD, mybir.AluOpType.max)
        else:
            nc.vector.tensor_scalar(mnext[:], cur_ps[:], biases[:, k:k + 1],
                                    0.0, ADD, mybir.AluOpType.max)
        ea = int(S.edges[a])
        ps_step = ppool_fix.tile([128, E], F32, tag="ps_step")
        nc.tensor.matmul(ps_step[:], w_ap(ea), mnext[:], start=True, stop=True)
        cur_ps = ps_step

    # res^T = ps_step + b_eroot  [d, e]
    eroot = int(S.edges[N - 1])
    nc.vector.tensor_scalar(mcur[:], cur_ps[:],
                            ebT_sb[:, eroot:eroot + 1], None, ADD)
    ps_sc = ppool_fix.tile([1, E + 4], F32, tag="ps_sc")
    nc.tensor.matmul(ps_sc[:, 0:E], scw_sb[:, 0:1], mcur[:],
                     start=True, stop=False)
    nc.tensor.matmul(ps_sc[:, E:E + 1], scw_sb[:, 1:2],
                     emb[:, POS_PURE_COL:POS_PURE_COL + 1], start=False, stop=True)
    dsc = pool.tile([1, 1], F32, tag="dsc")
    nc.vector.tensor_tensor(dsc[:], ps_sc[:, E:E + 1], sb_sb[:], ADD)
    srow = pool.tile([1, E], F32, tag="srow")
    nc.vector.tensor_scalar(srow[:], ps_sc[:, 0:E], dsc[:], None, ADD)
    nc.sync.dma_start(t_out[:], srow[:])


def _finalize(nc, emb16, ps, f):
    _, name, plo, phi, elo, ehi = f
    if phi <= plo:
        return
    nc.vector.tensor_scalar(emb16[:, elo:ehi], ps[name][:, plo:phi],
                            0.0, None, mybir.AluOpType.max)


# ----------------------------------------------------------------------------
# Host entry point
# ----------------------------------------------------------------------------

def kernel(**inputs):
    global LAST_RESULT
    data = np.asarray(inputs["data"])
    graphs = np.asarray(inputs["graphs"])
    edges = np.asarray(inputs["edges"])
    pos = int(np.asarray(inputs["pos"]))
    dv = np.asarray(inputs["data_vecs"], dtype=np.float32)
    dw = np.asarray(inputs["data_weights"], dtype=np.float32)
    db = np.asarray(inputs["data_biases"], dtype=np.float32)
    ew = np.asarray(inputs["edge_weights"], dtype=np.float32)
    eb = np.asarray(inputs["edge_biases"], dtype=np.float32)
    sew = np.asarray(inputs["score_embedding_weights"], dtype=np.float32)
    sdw = np.asarray(inputs["score_data_weights"], dtype=np.float32)
    sb = np.asarray(inputs["score_bias"], dtype=np.float32)

    scheds = [GraphSchedule(graphs[j], edges, pos) for j in range(G)]
    nc = _build_program(scheds)

    # ---- host-side data prep ----
    gv_rows = dv[data]                    # (N, VEC) gathered word vectors
    gv_pos = dv[data[pos]]
    dw_pad = np.zeros((VEC_PAD, D), np.float32)
    dw_pad[:VEC] = dw
    dw_pad[VEC] = db          # constant-1 input row folds the bias in
    dw_in = dw_pad.reshape(3, 128, D)

    shared = {
        "dw": dw_in.astype(np.float16),
        "eb": eb.astype(np.float16),
        "ebT": np.ascontiguousarray(eb.T),
        "db": db.reshape(D, 1),
        "scw": np.ascontiguousarray(np.concatenate([sew, sdw], axis=1)),
        "sb": sb.reshape(1, 1),
    }

    in_maps = []
    for j, S in enumerate(scheds):
        gvT = np.zeros((VEC_PAD, NCOLS), np.float16)
        for n in range(N):
            gvT[:VEC, S.col[n]] = gv_rows[n]
        gvT[:VEC, POS_PURE_COL] = gv_pos
        gvT[VEC, :POS_PURE_COL + 1] = 1.0
        wbuf = np.ascontiguousarray(
            ew[S.w_perm].transpose(1, 0, 2).reshape(D, E * D).astype(np.float16))
        m = dict(shared)
        m["gvT"] = gvT.reshape(3, 128, NCOLS)
        m["mult"] = S.mult.astype(np.float16)
        m["wbuf"] = wbuf
        in_maps.append(m)

    res = run_bass_kernel_spmd(nc, in_maps, core_ids=list(range(G)),
                               trace=bool(os.environ.get("BASS_TRACE")))
    LAST_RESULT = res
    out = np.stack([res.results[j]["scores"][0] for j in range(G)])
    return out.astype(np.float32)


# revision 18
# speedup vs baseline: 1.1792x; 1.0005x over previous
"""Trainium2 Bass kernel for nn_Net_49177375539428 (gnn_message_passing).

Strategy (see schedule builder below):
  - One core per candidate graph (8 graphs, 8 NeuronCores), single SPMD
    program with an 8-way switch on partition id; each branch is fully
    specialized to its graph's tree.
  - The (E,D) embedding matrices of the reference are row-constant except on
    the ancestor path of `pos`, so the computation decomposes into
      * a scalar chain: one vector x matrix transform per node (LDW + 1-col
        matmul accumulating straight into the parent's PSUM column,
        transposed layout [d, node]),
      * a branch at `pos` over all E edge matrices (1-col matmuls into a
        [d, e] PSUM tile, sharing weight loads with the chain),
      * ~log N full (D,D) matmuls along the pos->root path.
  - Edge weights are shipped once per core (8 MB) in first-use order and
    streamed through SBUF in chunks so DMA overlaps the PE work.
"""

import os
import numpy as np

import concourse.bass as bass
import concourse.mybir as mybir
import concourse.tile as tile
from concourse import bacc
from concourse.bass_utils import run_bass_kernel_spmd

N = 128          # nodes per graph
E = 128          # edge types
D = 128          # embedding dim
G = 8            # graphs / cores
VEC = 300        # word-vec dim
VEC_PAD = 384    # padded to 3x128
NCOLS = 132      # EMB columns: 128 nodes + pos_pure + pad to 4
POS_PURE_COL = 128
W_CHUNK_SLOTS = 32         # edge matrices per DMA chunk
N_W_CHUNKS = E // W_CHUNK_SLOTS

F32 = mybir.dt.float32
F16 = mybir.dt.float16

LAST_RESULT = None         # BassKernelResults of the most recent run


# ----------------------------------------------------------------------------
# Host-side schedule construction
# ----------------------------------------------------------------------------

class GraphSchedule:
    """Per-graph specialization: column assignment, wave structure, matmul
    schedule entries, and the per-core data (weight order, gvT, Mult)."""

    def __init__(self, g_row, edges, pos):
        parents = np.empty(N, np.int64)
        for i in range(N - 1):
            parents[i] = i + int(g_row[i])
        parents[N - 1] = -1
        children = [[] for _ in range(N)]
        for i in range(N - 1):
            children[parents[i]].append(i)
        internal = np.array([len(children[n]) > 0 for n in range(N)])
        depth = np.zeros(N, np.int64)
        for i in range(N - 2, -1, -1):
            depth[i] = depth[parents[i]] + 1
        maxdepth = int(depth.max())

        assert pos != N - 1, "pos == root not supported"
        path = []
        n = pos
        while n != N - 1:
            n = parents[n]
            path.append(n)
        pathset = set(path)

        # subtree of pos (incl. pos)
        sub = set()
        stack = [pos]
        while stack:
            n = stack.pop()
            sub.add(n)
            stack.extend(children[n])

        # Column assignment, level-major.  Within each level:
        #   [subtree-internal | other-internal | path nodes | leaves]
        col = np.full(N, -1, np.int64)
        self.sub_int_range = {}   # lvl -> (start, end)
        self.oth_int_range = {}   # lvl -> (start, end)
        off = 0
        lvl_nodes = [[] for _ in range(maxdepth + 1)]
        for n in range(N):
            lvl_nodes[depth[n]].append(n)
        for lvl in range(maxdepth + 1):
            nodes = lvl_nodes[lvl]
            sub_int = [n for n in nodes if n in sub and internal[n]]
            oth_int = [n for n in nodes
                       if internal[n] and n not in sub and n not in pathset]
            pth = [n for n in nodes if n in pathset]
            leaves = [n for n in nodes if not internal[n] and n not in pathset]
            self.sub_int_range[lvl] = (off, off + len(sub_int))
            for n in sub_int:
                col[n] = off
                off += 1
            self.oth_int_range[lvl] = (off, off + len(oth_int))
            for n in oth_int:
                col[n] = off
                off += 1
            for n in pth:
                col[n] = off
                off += 1
            for n in leaves:
                col[n] = off
                off += 1
        assert off == N

        self.parents, self.children = parents, children
        self.internal, self.depth, self.maxdepth = internal, depth, maxdepth
        self.path, self.pathset, self.sub = path, pathset, sub
        self.col = col
        self.pos = pos
        self.edges = edges
        self.path_idx = {a: k for k, a in enumerate(path)}
        # does path node have any chain (non-path, non-pos) children?
        self.path_has_chain = {
            a: any((c not in pathset) and c != pos for c in children[a])
            for a in path
        }

        self._build_entries()
        self._build_data_tables()

    def _build_entries(self):
        """Entries: (edge, [(src_col, psum_name, dst_col, start, stop)]).
        psum tiles: 'mini{lvl}', 'wave{lvl}', 'path', 'branch'."""
        edges, children, depth = self.edges, self.children, self.depth
        pos, sub, pathset = self.pos, self.sub, self.pathset
        col = self.col

        entries = []          # list of (edge_id, mm list)
        self.finalizes = []   # (after_entry_index, psum_name, psum_lo, psum_hi,
                              #  emb_lo, emb_hi)  -> EMB[lo:hi] = relu(psum+EMB)
        self.psum_sizes = {}

        # start/stop bookkeeping per (psum_name, dst_col)
        first_write = {}

        internal = self.internal
        pos_base = not internal[pos]

        def add_wave(kids_by_edge, psum_name, dst_of, branch_ok):
            """kids grouped per edge; appends entries (leaf-src edges first
            so they can overlap the previous wave's finalize)."""
            writer_cnt = {}
            for e, kids in kids_by_edge.items():
                for c in kids:
                    d = dst_of(c)
                    writer_cnt[d] = writer_cnt.get(d, 0) + 1
            seen_cnt = {}
            edge_order = sorted(kids_by_edge,
                                key=lambda e: (any(internal[c]
                                                   for c in kids_by_edge[e]), e))
            for e in edge_order:
                mms = []
                for c in kids_by_edge[e]:
                    d = dst_of(c)
                    seen_cnt[d] = seen_cnt.get(d, 0) + 1
                    key = (psum_name, d)
                    start = key not in first_write
                    first_write[key] = True
                    stop = seen_cnt[d] == writer_cnt[d]
                    mms.append((col[c], not internal[c],
                                psum_name, d, start, stop))
                if branch_ok and not self.branch_done[e]:
                    self.branch_done[e] = True
                    mms.append((col[pos], pos_base, 'branch', e, True, True))
                entries.append(('W', e, mms))

        self.branch_done = [False] * E

        # --- mini waves: subtree of pos, deepest level first -----------------
        sub_lvls = sorted({int(depth[n]) for n in sub}, reverse=True)
        for lvl in sub_lvls:
            # children at this level whose parents are in sub at lvl-1
            kids = [n for n in sub if depth[n] == lvl and n != pos]
            if kids:
                slo, shi = self.sub_int_range[lvl - 1]
                entries.append(('B', f'mini{lvl}', slo, shi))
                by_edge = {}
                for c in kids:
                    by_edge.setdefault(int(edges[c]), []).append(c)
                add_wave(by_edge, f'mini{lvl}',
                         lambda c: col[self.parents[c]]
                         - self.sub_int_range[int(depth[c]) - 1][0],
                         branch_ok=False)
                lo, hi = self.sub_int_range[lvl - 1]
                self.psum_sizes[f'mini{lvl}'] = hi - lo
                self.finalizes.append((len(entries), f'mini{lvl}',
                                       0, hi - lo, lo, hi))
        # v_pos is now available (pos is leaf, or finalized by last mini wave)

        # --- main waves ------------------------------------------------------
        main_edges = set()
        for lvl in range(self.maxdepth, 0, -1):
            for n in range(N):
                if depth[n] == lvl and n not in sub and n not in pathset:
                    main_edges.add(int(edges[n]))
        leftover = [e for e in range(E) if e not in main_edges]
        n_main = sum(1 for lvl in range(self.maxdepth, 0, -1)
                     if any(depth[n] == lvl and n not in sub and n not in pathset
                            for n in range(N)))
        per_wave = (len(leftover) + max(n_main, 1) - 1) // max(n_main, 1)
        lq = list(leftover)
        for lvl in range(self.maxdepth, 0, -1):
            kids = [n for n in range(N)
                    if depth[n] == lvl and n not in sub and n not in pathset]
            olo, ohi = self.oth_int_range[lvl - 1]
            if ohi > olo:
                entries.append(('B', f'wave{lvl - 1}', olo, ohi))
            if kids:
                by_edge = {}
                for c in kids:
                    by_edge.setdefault(int(edges[c]), []).append(c)

                def dst_of(c):
                    p = self.parents[c]
                    if p in pathset:
                        return ('path', self.path_idx[p])
                    return (f'wave{lvl - 1}', col[p] - olo)

                wcnt = {}
                for e, kids_e in by_edge.items():
                    for c in kids_e:
                        tgt = dst_of(c)
                        wcnt[tgt] = wcnt.get(tgt, 0) + 1
                seen = {}
                edge_order = sorted(by_edge,
                                    key=lambda e: (any(internal[c]
                                                       for c in by_edge[e]), e))
                for e in edge_order:
                    mms = []
                    for c in by_edge[e]:
                        name, d = dst_of(c)
                        tgt = (name, d)
                        seen[tgt] = seen.get(tgt, 0) + 1
                        start = tgt not in first_write
                        first_write[tgt] = True
                        stop = seen[tgt] == wcnt[tgt]
                        mms.append((col[c], not internal[c], name, d, start, stop))
                    if not self.branch_done[e]:
                        self.branch_done[e] = True
                        mms.append((col[pos], pos_base, 'branch', e, True, True))
                    entries.append(('W', e, mms))
                for _ in range(per_wave):
                    if not lq:
                        break
                    e = lq.pop()
                    self.branch_done[e] = True
                    entries.append(('W', e, [(col[pos], pos_base,
                                              'branch', e, True, True)]))
            if ohi > olo:
                self.psum_sizes[f'wave{lvl - 1}'] = ohi - olo
                self.finalizes.append((len(entries), f'wave{lvl - 1}',
                                       0, ohi - olo, olo, ohi))

        # --- leftover branch edges ------------------------------------------
        for e in range(E):
            if not self.branch_done[e]:
                self.branch_done[e] = True
                entries.append(('W', e, [(self.col[pos], pos_base,
                                          'branch', e, True, True)]))

        self.psum_sizes['branch'] = E
        self.psum_sizes['path'] = max(1, len(self.path))

        # PSUM start/stop semantics: start=True lazily zeroes the ENTIRE
        # 2KB bank (pending-zero), after which the per-byte pending flag
        # makes fresh columns overwrite and touched columns accumulate.
        # So: start only on the very first matmul into each tile, stop on
        # the last.  (Per-column start flags would wipe sibling columns.)
        totals = {}
        for ent in entries:
            if ent[0] == 'B':
                totals[ent[1]] = totals.get(ent[1], 0) + 4
            else:
                for (_, _, pname, _, _, _) in ent[2]:
                    totals[pname] = totals.get(pname, 0) + 1
        seen = {}
        fixed = []
        for ent in entries:
            if ent[0] == 'B':
                _, pname, lo, hi = ent
                k = seen.get(pname, 0)
                seen[pname] = k + 4
                fixed.append(('B', pname, lo, hi, k == 0,
                              k + 4 == totals[pname]))
                continue
            _, e, mms = ent
            new_mms = []
            for (src, sbase, pname, dst, _, _) in mms:
                k = seen.get(pname, 0)
                seen[pname] = k + 1
                new_mms.append((src, sbase, pname, dst,
                                k == 0, k + 1 == totals[pname]))
            fixed.append(('W', e, new_mms))
        self.entries = fixed

        # Wbuf slot order = first use
        slot_of = {}
        for ent in entries:
            if ent[0] != 'W':
                continue
            e = ent[1]
            if e not in slot_of:
                slot_of[e] = len(slot_of)
        assert len(slot_of) == E
        self.slot_of = slot_of

    def _build_data_tables(self):
        """Per-core numpy inputs: Wbuf (first-use order), Mult, perm for gvT."""
        perm = np.empty(E, np.int64)     # slot -> edge id
        for e, s in self.slot_of.items():
            perm[s] = e
        self.w_perm = perm

        # multiplicity matrix: Mult[e, col(p)] = # chain children of p with edge e
        mult = np.zeros((E, NCOLS), np.float32)
        for p in range(N):
            for c in self.children[p]:
                if c in self.pathset or c == self.pos:
                    continue
                mult[int(self.edges[c]), self.col[p]] += 1.0
        self.mult = mult


# ----------------------------------------------------------------------------
# Bass program
# ----------------------------------------------------------------------------

def _build_program(scheds):
    nc = bacc.Bacc("TRN2", target_bir_lowering=False, debug=False, num_devices=G)

    # shared inputs
    t_dw = nc.declare_dram_parameter("dw", [3, 128, D], F16, isOutput=False)
    t_eb = nc.declare_dram_parameter("eb", [E, D], F16, isOutput=False)
    t_ebT = nc.declare_dram_parameter("ebT", [D, E], F32, isOutput=False)
    t_db = nc.declare_dram_parameter("db", [D, 1], F32, isOutput=False)
    t_scw = nc.declare_dram_parameter("scw", [D, 2], F32, isOutput=False)
    t_sb = nc.declare_dram_parameter("sb", [1, 1], F32, isOutput=False)
    # per-core inputs
    t_gvT = nc.declare_dram_parameter("gvT", [3, 128, NCOLS], F16, isOutput=False)
    t_mult = nc.declare_dram_parameter("mult", [E, NCOLS], F16, isOutput=False)
    t_w = nc.declare_dram_parameter("wbuf", [D, E * D], F16, isOutput=False)
    t_out = nc.declare_dram_parameter("scores", [1, E], F32, isOutput=True)

    with tile.TileContext(nc) as tc:
        with (
            tc.tile_pool(name="wpool", bufs=1) as wpool,
            tc.tile_pool(name="sbuf", bufs=1) as pool,
            tc.tile_pool(name="ppool", bufs=2, space="PSUM") as ppool,
            tc.tile_pool(name="ppool_fix", bufs=1, space="PSUM") as ppool_fix,
        ):
            # All DMA loads are identical instructions across graphs (per-core
            # content differs via in_maps) — issue them before the Switch so
            # transfers stream from t=0, overlapping dispatch + compute.
            # W chunks first, all on the sync HWDGE: triggers land at t~0
            # and the hardware DGE streams the 4.2MB autonomously.
            wb = t_w.ap().rearrange("p (c s) -> p c s", c=N_W_CHUNKS)
            w_chunks = []
            for c in range(N_W_CHUNKS):
                wt = wpool.tile([D, W_CHUNK_SLOTS * D], F16, tag=f"w{c}",
                                name=f"w{c}")
                nc.sync.dma_start(wt[:], wb[:, c, :])
                w_chunks.append(wt)
            sb_tiles = {}
            sb_tiles['dw'] = pool.tile([128, 3, D], F16, tag="dw", name="dw")
            nc.scalar.dma_start(sb_tiles['dw'][:], t_dw.ap().rearrange("c p d -> p c d"))
            sb_tiles['gv'] = pool.tile([128, 3, NCOLS], F16, tag="gv", name="gv")
            nc.scalar.dma_start(sb_tiles['gv'][:], t_gvT.ap().rearrange("c p d -> p c d"))
            sb_tiles['mult'] = pool.tile([E, NCOLS], F16, tag="mult", name="mult")
            nc.scalar.dma_start(sb_tiles['mult'][:], t_mult[:])
            sb_tiles['eb'] = pool.tile([E, D], F16, tag="eb", name="eb")
            nc.scalar.dma_start(sb_tiles['eb'][:], t_eb[:])
            sb_tiles['ebT'] = pool.tile([D, E], F32, tag="ebT", name="ebT")
            nc.scalar.dma_start(sb_tiles['ebT'][:], t_ebT[:])
            sb_tiles['db'] = pool.tile([D, 1], F32, tag="db", name="db")
            nc.scalar.dma_start(sb_tiles['db'][:], t_db[:])
            sb_tiles['scw'] = pool.tile([D, 2], F32, tag="scw", name="scw")
            nc.scalar.dma_start(sb_tiles['scw'][:], t_scw[:])
            sb_tiles['sb'] = pool.tile([1, 1], F32, tag="sb", name="sb")
            nc.scalar.dma_start(sb_tiles['sb'][:], t_sb[:])
            pid = nc.partition_id()
            for j in tc.Switch(pid, G):
                _emit_graph(nc, scheds[j], pool, ppool, ppool_fix,
                            sb_tiles, w_chunks, t_out)
    nc.finalize()
    return nc


def _emit_graph(nc, S, pool, ppool, ppool_fix, sb_tiles, w_chunks, t_out):
    Relu = mybir.ActivationFunctionType.Relu
    ADD = mybir.AluOpType.add
    dw_sb = sb_tiles['dw']
    eb_sb = sb_tiles['eb']
    ebT_sb = sb_tiles['ebT']
    db_sb = sb_tiles['db']
    scw_sb = sb_tiles['scw']
    sb_sb = sb_tiles['sb']
    gv_sb = sb_tiles['gv']
    mult_sb = sb_tiles['mult']

    def w_ap(edge):
        s = S.slot_of[edge]
        return w_chunks[s // W_CHUNK_SLOTS][
            :, (s % W_CHUNK_SLOTS) * D:(s % W_CHUNK_SLOTS + 1) * D]

    # ---- EMB = baseB (base + bias-sums), then finalized in-place ----
    ps_base = ppool_fix.tile([128, NCOLS], F32, tag="ps_base")
    for k in range(3):
        nc.tensor.matmul(ps_base[:], dw_sb[:, k, :], gv_sb[:, k, :],
                         start=(k == 0), stop=False)
    nc.tensor.matmul(ps_base[:], eb_sb[:], mult_sb[:], start=False, stop=True)
    emb = pool.tile([128, NCOLS], F32, tag="emb")
    nc.vector.tensor_copy(emb[:], ps_base[:])
    emb16b = pool.tile([128, NCOLS], F16, tag="emb16b")   # static: leaves etc.
    nc.vector.tensor_copy(emb16b[:], emb[:])
    emb16 = pool.tile([128, NCOLS], F16, tag="emb16")     # finalize targets

    # ---- psum tiles for waves / branch / path ----
    ps = {}
    ps['branch'] = ppool_fix.tile([128, E], F32, tag="ps_branch", name="ps_branch")
    ps['path'] = ppool_fix.tile([128, S.psum_sizes['path']], F32, tag="ps_path", name="ps_path")
    for name, sz in S.psum_sizes.items():
        if name in ('branch', 'path'):
            continue
        ps[name] = ppool.tile([128, sz], F32, tag="ps_wave", name=f"ps_{name}")

    # ---- chain + branch matmuls with interleaved finalizes ----
    fin = list(S.finalizes)
    fi = 0
    for idx, ent in enumerate(S.entries):
        while fi < len(fin) and fin[fi][0] == idx:
            _finalize(nc, emb16, ps, fin[fi])
            fi += 1
        if ent[0] == 'B':
            _, pname, lo, hi, start, stop = ent
            pt = ps[pname]
            w = hi - lo
            for k in range(3):
                nc.tensor.matmul(pt[:, 0:w], dw_sb[:, k, :],
                                 gv_sb[:, k, lo:hi],
                                 start=(start and k == 0), stop=False)
            nc.tensor.matmul(pt[:, 0:w], eb_sb[:], mult_sb[:, lo:hi],
                             start=False, stop=stop)
            continue
        _, e, mms = ent
        wap = w_ap(e)
        for (src, sbase, pname, dst, start, stop) in mms:
            mv = emb16b if sbase else emb16
            nc.tensor.matmul(ps[pname][:, dst:dst + 1], wap,
                             mv[:, src:src + 1], start=start, stop=stop)
    while fi < len(fin):
        _finalize(nc, emb16, ps, fin[fi])
        fi += 1

    # ---- path walk ----
    # step k at ancestor a_k:  M_k = relu(prev_transform + b_{e_{k-1}} + u_k)
    # with u_k = baseB[a_k] (+ chain psum).  Bias columns are precomputed so
    # each step is ONE fused DVE op (add per-partition bias, max 0) + one MM.
    plen = len(S.path)
    biases = pool.tile([128, max(plen, 1)], F32, tag="biases")
    for k, a in enumerate(S.path):
        acol = int(S.col[a])
        eprev = None if k == 0 else int(S.edges[S.path[k - 1]])
        if S.path_has_chain[a]:
            nc.vector.tensor_tensor(
                biases[:, k:k + 1], ps['path'][:, k:k + 1],
                emb[:, acol:acol + 1], ADD)
            if eprev is not None:
                nc.vector.tensor_tensor(
                    biases[:, k:k + 1], biases[:, k:k + 1],
                    ebT_sb[:, eprev:eprev + 1], ADD)
        elif eprev is not None:
            nc.vector.tensor_tensor(
                biases[:, k:k + 1], emb[:, acol:acol + 1],
                ebT_sb[:, eprev:eprev + 1], ADD)
        else:
            nc.vector.tensor_copy(biases[:, k:k + 1], emb[:, acol:acol + 1])

    mcur = pool.tile([128, E], F32, tag="mcur")
    mnext = pool.tile([128, E], F16, tag="mnext")
    cur_ps = ps['branch']
    for k, a in enumerate(S.path):
        if k == 0:
            # branch contribution carries full per-column edge biases
            nc.vector.tensor_tensor(mcur[:], cur_ps[:], ebT_sb[:], ADD)
            nc.vector.tensor_scalar(mnext[:], mcur[:], biases[:, 0:1], 0.0,
                                    ADD, mybir.AluOpType.max)
        else:
            nc.vector.tensor_scalar(mnext[:], cur_ps[:], biases[:, k:k + 1],
                                    0.0, ADD, mybir.AluOpType.max)
        ea = int(S.edges[a])
        ps_step = ppool_fix.tile([128, E], F32, tag="ps_step")
        nc.tensor.matmul(ps_step[:], w_ap(ea), mnext[:], start=True, stop=True)
        cur_ps = ps_step

    # res^T = ps_step + b_eroot  [d, e]
    eroot = int(S.edges[N - 1])
    nc.vector.tensor_scalar(mcur[:], cur_ps[:],
                            ebT_sb[:, eroot:eroot + 1], None, ADD)
    ps_sc = ppool_fix.tile([1, E + 4], F32, tag="ps_sc")
    nc.tensor.matmul(ps_sc[:, 0:E], scw_sb[:, 0:1], mcur[:],
                     start=True, stop=False)
    nc.tensor.matmul(ps_sc[:, E:E + 1], scw_sb[:, 1:2],
                     emb[:, POS_PURE_COL:POS_PURE_COL + 1], start=False, stop=True)
    dsc = pool.tile([1, 1], F32, tag="dsc")
    nc.vector.tensor_tensor(dsc[:], ps_sc[:, E:E + 1], sb_sb[:], ADD)
    srow = pool.tile([1, E], F32, tag="srow")
    nc.vector.tensor_scalar(srow[:], ps_sc[:, 0:E], dsc[:], None, ADD)
    nc.sync.dma_start(t_out[:], srow[:])


def _finalize(nc, emb16, ps, f):
    _, name, plo, phi, elo, ehi = f
    if phi <= plo:
        return
    nc.vector.tensor_scalar(emb16[:, elo:ehi], ps[name][:, plo:phi],
                            0.0, None, mybir.AluOpType.max)


# ----------------------------------------------------------------------------
# Host entry point
# ----------------------------------------------------------------------------

def kernel(**inputs):
    global LAST_RESULT
    data = np.asarray(inputs["data"])
    graphs = np.asarray(inputs["graphs"])
    edges = np.asarray(inputs["edges"])
    pos = int(np.asarray(inputs["pos"]))
    dv = np.asarray(inputs["data_vecs"], dtype=np.float32)
    dw = np.asarray(inputs["data_weights"], dtype=np.float32)
    db = np.asarray(inputs["data_biases"], dtype=np.float32)
    ew = np.asarray(inputs["edge_weights"], dtype=np.float32)
    eb = np.asarray(inputs["edge_biases"], dtype=np.float32)
    sew = np.asarray(inputs["score_embedding_weights"], dtype=np.float32)
    sdw = np.asarray(inputs["score_data_weights"], dtype=np.float32)
    sb = np.asarray(inputs["score_bias"], dtype=np.float32)

    scheds = [GraphSchedule(graphs[j], edges, pos) for j in range(G)]
    nc = _build_program(scheds)

    # ---- host-side data prep ----
    gv_rows = dv[data]                    # (N, VEC) gathered word vectors
    gv_pos = dv[data[pos]]
    dw_pad = np.zeros((VEC_PAD, D), np.float32)
    dw_pad[:VEC] = dw
    dw_pad[VEC] = db          # constant-1 input row folds the bias in
    dw_in = dw_pad.reshape(3, 128, D)

    shared = {
        "dw": dw_in.astype(np.float16),
        "eb": eb.astype(np.float16),
        "ebT": np.ascontiguousarray(eb.T),
        "db": db.reshape(D, 1),
        "scw": np.ascontiguousarray(np.concatenate([sew, sdw], axis=1)),
        "sb": sb.reshape(1, 1),
    }

    in_maps = []
    for j, S in enumerate(scheds):
        gvT = np.zeros((VEC_PAD, NCOLS), np.float16)
        for n in range(N):
            gvT[:VEC, S.col[n]] = gv_rows[n]
        gvT[:VEC, POS_PURE_COL] = gv_pos
        gvT[VEC, :POS_PURE_COL + 1] = 1.0
        wbuf = np.ascontiguousarray(
            ew[S.w_perm].transpose(1, 0, 2).reshape(D, E * D).astype(np.float16))
        m = dict(shared)
        m["gvT"] = gvT.reshape(3, 128, NCOLS)
        m["mult"] = S.mult.astype(np.float16)
        m["wbuf"] = wbuf
        in_maps.append(m)

    res = run_bass_kernel_spmd(nc, in_maps, core_ids=list(range(G)),
                               trace=bool(os.environ.get("BASS_TRACE")))
    LAST_RESULT = res
    out = np.stack([res.results[j]["scores"][0] for j in range(G)])
    return out.astype(np.float32)
